# revision 1
# baseline (speedup 1.0000x reference)
"""Trainium2 Bass kernel for nn_Attention_17179869826 (GIN structure extractor +
masked dense attention over a ragged batch of graphs).

Sharding: data-parallel over graphs — 4 graphs per core, each padded to L=512
rows (R=2048 padded rows per core).  All heavy compute runs on the PE array in
float32r (full-rate fp32).  The only cross-core communication is a tiny
AllReduce of the BatchNorm sum/sumsq statistics.

Layouts:
  rm  = row-major   [rows on partitions, features on free axis]
  fm  = feature-major [features on partitions, rows on free axis]

Per-core pipeline (all matmuls = PE, evictions = ACT/DVE):
  v      = x @ v_w + v_b                  (Form-X: lhsT=xT, rhs=W)   -> DRAM
  GIN l=0,1: zT=AGG-Z'(h_rm,C+I) fm; uT=relu(W1ᵀzT+b1) fm; h'=relu(uT ᵀW2+b2) rm
  GIN l=2  : same but h3T produced fm (Form-Y)
  BN: valid-masked sum/sumsq per feature -> AllReduce(8) -> affine in fm
  x_structT = W_seᵀ h3T + b (fm);  qkT = W_qkᵀ x_structT + b (fm, per graph)
  attention per (graph, head): dotsT[j,i] = kᵀq + maskT (PE-accumulated mask),
  eT=exp (ACT), U = Vᵀ eT and den = 1ᵀ eT (same PSUM), recip via approx-NR,
  out = (U*recip)ᵀ-projected per head with out_w, + out_b.
"""

import functools
import sys

import numpy as np

sys.path.insert(0, "/opt/trn_rl_repo")

import concourse.bass as bass
import concourse.bacc as bacc
import concourse.mybir as mybir
from concourse import tile
from concourse.bass_utils import run_bass_kernel_spmd

F32 = mybir.dt.float32
F32R = mybir.dt.float32r
AF = mybir.ActivationFunctionType
ALU = mybir.AluOpType

NCORE = 8
B = 32
L = 512
D = 512
H = 8
HD = 64
GPC = B // NCORE          # graphs per core = 4
R = GPC * L               # padded rows per core = 2048
NT = D // 128             # feature sub-tiles = 4
NRT = R // 128            # row sub-tiles = 16
NEG = -1.0e30

# bpack column layout ([128, 40]) — per-partition biases for fm evictions
BP_B1 = 0        # gin_b1[l] at cols 4l+m   (l=0..2)
BP_B2L2 = 12     # gin_b2[2]
BP_SE = 16       # se_out_b
BP_QK = 20       # qk_b (8 cols, q-half pre-scaled)
BP_GAMMA = 28
BP_BETA = 32
BP_NCOL = 40

# brow slots ([1, 4, 512]) — row-vector biases added via K=1 matmuls
BR_VB = 0
BR_B2L0 = 1
BR_B2L1 = 2
BR_OUTB = 3


def _r(ap):
    return ap.bitcast(F32R)


def _build():
    nc = bacc.Bacc(None, target_bir_lowering=False)

    di = {}
    def inp(name, shape, dt=F32R):
        di[name] = nc.dram_tensor(name, list(shape), dt, kind="ExternalInput")
        return di[name]

    x_rm = inp("x_rm", (R, D))
    xT = inp("xT", (D, R))
    cp = inp("cp", (R, L))          # (C + I) per graph, rows = in-core src
    mT = inp("mT", (R, L))          # additive mask, transposed: rows = key j
    wg1 = inp("wg1", (3, D, D))
    wg2 = inp("wg2", (3, D, D))
    wse = inp("wse", (D, D))
    wqk = inp("wqk", (D, 2 * D))
    wv = inp("wv", (D, D))
    wo = inp("wo", (D, D))
    brow = inp("brow", (1, 4, D))
    bpack = inp("bpack", (128, BP_NCOL), F32)
    valid = inp("valid", (128, R), F32)
    ident = inp("ident", (128, 128))
    onem = inp("onem", (128, 128))

    y = nc.dram_tensor("y", [R, D], F32, kind="ExternalOutput")

    inv_n = None  # baked after we know N; set by caller via closure? -> passed in
    return nc, di, y


def _emit(nc, di, y, inv_n, zflags):
    x_rm, xT, cp, mT = di["x_rm"], di["xT"], di["cp"], di["mT"]
    wg1, wg2, wse, wqk, wv, wo = (di[k] for k in ("wg1", "wg2", "wse", "wqk", "wv", "wo"))
    brow_d, bpack_d, valid_d, ident_d, onem_d = (
        di[k] for k in ("brow", "bpack", "valid", "ident", "onem"))

    with tile.TileContext(nc) as tc:
        # ---- pools ------------------------------------------------------
        # long-lived pools on the LEFT side of the SBUF stack allocator;
        # phase-transient pools on the RIGHT (released in reverse order).
        pcon = tc.alloc_tile_pool(name="con", bufs=1, side="left")
        pp = tc.alloc_tile_pool(name="ps", bufs=8, space="PSUM")
        pd = tc.alloc_tile_pool(name="dram", bufs=4, space="DRAM")

        # ---- constants --------------------------------------------------
        ct_ident = pcon.tile([128, 128], F32R, tag="ident", name="t_ident")
        ct_ones = pcon.tile([128, 128], F32R, tag="ones", name="t_ones")
        ct_bpack = pcon.tile([128, BP_NCOL], F32, tag="bpack", name="t_bpack")
        ct_brow = pcon.tile([1, 4, D], F32R, tag="brow", name="t_brow")
        nc.sync.dma_start(out=ct_ident[:], in_=ident_d[:])
        nc.sync.dma_start(out=ct_ones[:], in_=onem_d[:])
        nc.sync.dma_start(out=ct_bpack[:], in_=bpack_d[:])
        nc.sync.dma_start(out=ct_brow[:], in_=brow_d[:])

        v_dram = pd.tile([R, D], F32R, tag="vd", name="t_vdram")
        cc_in = pd.tile([128, 2, NT], F32, tag="cci", name="t_ccin")
        cc_out = pd.tile([128, 2, NT], F32, tag="cco", name="t_ccout")

        def load_w(dram_ap, nm):
            t = pw.tile([128, NT, D], F32R, tag="w", name=nm)
            nc.sync.dma_start(
                out=t[:], in_=dram_ap.rearrange("(kt p) n -> p kt n", p=128))
            return t

        def bias_mm(ps_ap, slot):
            # += ones[1,128].T @ brow[1,512]  (adds a row-vector bias to all rows)
            if zflags[slot]:
                return  # bias is identically zero in the input data
            nc.tensor.matmul(
                out=ps_ap, lhsT=_r(ct_ones[0:1, 0:128]),
                rhs=_r(ct_brow[0:1, slot, :]),
                start=False, stop=True, skip_group_check=True)

        # ================= phase V: v = x @ wv + vb -> v_dram =============
        pw = tc.alloc_tile_pool(name="w", bufs=2, side="left")
        pxs = tc.alloc_tile_pool(name="xs", bufs=NT, side="left")
        ph3 = tc.alloc_tile_pool(name="h3", bufs=NT, side="left")
        ph = tc.alloc_tile_pool(name="h", bufs=4, side="right")
        pC = tc.alloc_tile_pool(name="C", bufs=2, side="right")
        pxT = tc.alloc_tile_pool(name="xT", bufs=NT, side="right")
        pvst = tc.alloc_tile_pool(name="vst", bufs=2, side="right")

        wv_t = load_w(wv[:], "t_wv")
        xT_t = []
        for t in range(NT):
            tt = pxT.tile([128, R], F32R, tag="xT", name=f"t_xT{t}")
            nc.sync.dma_start(out=tt[:], in_=xT[128 * t:128 * (t + 1), :])
            xT_t.append(tt)
        # issue the GIN input loads now so they stream behind the V matmuls
        h_t = []
        for g in range(GPC):
            tt = ph.tile([128, NT, L], F32R, tag="h", name=f"h0_{g}")
            nc.sync.dma_start(
                out=tt[:],
                in_=x_rm[L * g:L * (g + 1), :].rearrange("(st p) n -> p st n", p=128))
            h_t.append(tt)
        for rt in range(NRT):
            ps = pp.tile([128, D], F32, tag="ps", name=f"ps_v{rt}")
            for kt in range(NT):
                nc.tensor.matmul(
                    out=ps[:], lhsT=_r(xT_t[kt][:, 128 * rt:128 * (rt + 1)]),
                    rhs=_r(wv_t[:, kt, :]),
                    start=(kt == 0), stop=False, skip_group_check=True)
            bias_mm(ps[:], BR_VB)
            vs = pvst.tile([128, D], F32R, tag="vst", name=f"vs{rt}")
            nc.any.tensor_copy(out=vs[:], in_=ps[:])
            nc.sync.dma_start(out=v_dram[128 * rt:128 * (rt + 1), :], in_=vs[:])
        pvst.release()
        pxT.release()

        # ================= GIN layers ====================================
        pz = tc.alloc_tile_pool(name="z", bufs=NT, side="right")
        pu = tc.alloc_tile_pool(name="u", bufs=NT, side="right")

        h3_t = []
        for lay in range(3):
            w1_t = load_w(wg1[lay], f"t_w1_{lay}")
            w2_t = load_w(wg2[lay], f"t_w2_{lay}")
            z_t = [pz.tile([128, R], F32R, tag="z", name=f"z{lay}_{d}") for d in range(NT)]
            u_t = [pu.tile([128, R], F32R, tag="u", name=f"u{lay}_{d}") for d in range(NT)]
            # --- AGG: zT[d, 512g+j] = sum_s h[s, d] * C'[s, j] ---
            for g in range(GPC):
                cg = pC.tile([128, NT, L], F32R, tag="C", name=f"c{lay}_{g}")
                nc.sync.dma_start(
                    out=cg[:],
                    in_=cp[L * g:L * (g + 1), :].rearrange("(st p) n -> p st n", p=128))
                for dt in range(NT):
                    ps = pp.tile([128, L], F32, tag="ps", name=f"ps_z{lay}_{g}_{dt}")
                    for st in range(NT):
                        nc.tensor.matmul(
                            out=ps[:],
                            lhsT=_r(h_t[g][:, st, 128 * dt:128 * (dt + 1)]),
                            rhs=_r(cg[:, st, :]),
                            start=(st == 0), stop=(st == NT - 1),
                            skip_group_check=True)
                    nc.any.tensor_copy(out=z_t[dt][:, L * g:L * (g + 1)], in_=ps[:])
            # --- MLP1: uT = relu(W1.T @ zT + b1) (fm) ---
            for mt in range(NT):
                for c in range(GPC):
                    ps = pp.tile([128, L], F32, tag="ps", name=f"ps_u{lay}_{mt}_{c}")
                    for kt in range(NT):
                        nc.tensor.matmul(
                            out=ps[:],
                            lhsT=_r(w1_t[:, kt, 128 * mt:128 * (mt + 1)]),
                            rhs=_r(z_t[kt][:, L * c:L * (c + 1)]),
                            start=(kt == 0), stop=(kt == NT - 1),
                            skip_group_check=True)
                    nc.scalar.activation(
                        out=u_t[mt][:, L * c:L * (c + 1)], in_=ps[:], func=AF.Relu,
                        bias=ct_bpack[:, 4 * lay + mt:4 * lay + mt + 1])
            if lay < 2:
                # --- MLP2 Form-X: h'[row, dout] = relu(uT.T @ W2 + b2) (rm) ---
                hn = [ph.tile([128, NT, L], F32R, tag="h", name=f"h{lay+1}_{g}")
                      for g in range(GPC)]
                for rt in range(NRT):
                    ps = pp.tile([128, D], F32, tag="ps", name=f"ps_h{lay}_{rt}")
                    for kt in range(NT):
                        nc.tensor.matmul(
                            out=ps[:],
                            lhsT=_r(u_t[kt][:, 128 * rt:128 * (rt + 1)]),
                            rhs=_r(w2_t[:, kt, :]),
                            start=(kt == 0), stop=False, skip_group_check=True)
                    bias_mm(ps[:], BR_B2L0 + lay)
                    nc.scalar.activation(
                        out=hn[rt // NT][:, rt % NT, :], in_=ps[:], func=AF.Relu)
                h_t = hn
            else:
                # --- MLP2 Form-Y: h3T = relu(W2.T @ uT + b2) (fm) ---
                for mt in range(NT):
                    t3 = ph3.tile([128, R], F32R, tag="h3", name=f"h3_{mt}")
                    h3_t.append(t3)
                for mt in range(NT):
                    for c in range(GPC):
                        ps = pp.tile([128, L], F32, tag="ps", name=f"ps_h3_{mt}_{c}")
                        for kt in range(NT):
                            nc.tensor.matmul(
                                out=ps[:],
                                lhsT=_r(w2_t[:, kt, 128 * mt:128 * (mt + 1)]),
                                rhs=_r(u_t[kt][:, L * c:L * (c + 1)]),
                                start=(kt == 0), stop=(kt == NT - 1),
                                skip_group_check=True)
                        nc.scalar.activation(
                            out=h3_t[mt][:, L * c:L * (c + 1)], in_=ps[:],
                            func=AF.Relu,
                            bias=ct_bpack[:, BP_B2L2 + mt:BP_B2L2 + mt + 1])
        pu.release()
        pz.release()
        pC.release()
        ph.release()

        # ================= BatchNorm (global over all cores) ==============
        pbn = tc.alloc_tile_pool(name="bn", bufs=1, side="right")
        psq = tc.alloc_tile_pool(name="sq", bufs=2, side="right")
        stats = pbn.tile([128, 2, NT], F32, tag="st", name="t_stats")
        ct_valid = pbn.tile([128, R], F32, tag="valid", name="t_valid")
        nc.sync.dma_start(out=ct_valid[:], in_=valid_d[:])
        for mt in range(NT):
            nc.vector.tensor_mul(out=h3_t[mt][:], in0=h3_t[mt][:], in1=ct_valid[:])
            sq = psq.tile([128, R], F32, tag="sq", name=f"sq{mt}")
            nc.scalar.activation(out=sq[:], in_=h3_t[mt][:], func=AF.Square)
            nc.vector.tensor_reduce(
                out=stats[:, 0, mt:mt + 1], in_=h3_t[mt][:],
                axis=mybir.AxisListType.X, op=ALU.add)
            nc.vector.tensor_reduce(
                out=stats[:, 1, mt:mt + 1], in_=sq[:],
                axis=mybir.AxisListType.X, op=ALU.add)
        nc.sync.dma_start(out=cc_in[:], in_=stats[:])
        nc.gpsimd.collective_compute(
            "AllReduce", ALU.add, replica_groups=[list(range(NCORE))],
            ins=[cc_in.opt()], outs=[cc_out.opt()])
        gstats = pbn.tile([128, 2, NT], F32, tag="gst", name="t_gstats")
        nc.sync.dma_start(out=gstats[:], in_=cc_out[:])

        bnm = pbn.tile([128, NT], F32, tag="bnm", name="t_bnm")
        bne = pbn.tile([128, NT], F32, tag="bne", name="t_bne")
        bnv = pbn.tile([128, NT], F32, tag="bnv", name="t_bnv")
        bns = pbn.tile([128, NT], F32, tag="bns", name="t_bns")
        bni = pbn.tile([128, NT], F32, tag="bni", name="t_bni")
        bna = pbn.tile([128, NT], F32, tag="bna", name="t_bna")
        bnb = pbn.tile([128, NT], F32, tag="bnb", name="t_bnb")
        nc.vector.tensor_scalar_mul(out=bnm[:], in0=gstats[:, 0, :], scalar1=inv_n)
        nc.vector.tensor_scalar_mul(out=bne[:], in0=gstats[:, 1, :], scalar1=inv_n)
        nc.vector.tensor_mul(out=bnv[:], in0=bnm[:], in1=bnm[:])
        nc.vector.tensor_sub(out=bnv[:], in0=bne[:], in1=bnv[:])
        nc.vector.tensor_scalar_add(out=bnv[:], in0=bnv[:], scalar1=1e-5)
        nc.scalar.activation(out=bns[:], in_=bnv[:], func=AF.Sqrt)
        nc.vector.reciprocal(out=bni[:], in_=bns[:])
        nc.vector.tensor_mul(out=bna[:], in0=ct_bpack[:, BP_GAMMA:BP_GAMMA + NT], in1=bni[:])
        nc.vector.tensor_mul(out=bnb[:], in0=bnm[:], in1=bna[:])
        nc.vector.tensor_sub(out=bnb[:], in0=ct_bpack[:, BP_BETA:BP_BETA + NT], in1=bnb[:])
        for mt in range(NT):
            nc.vector.tensor_scalar(
                out=h3_t[mt][:], in0=h3_t[mt][:],
                scalar1=bna[:, mt:mt + 1], scalar2=bnb[:, mt:mt + 1],
                op0=ALU.mult, op1=ALU.add)
        psq.release()
        pbn.release()

        # ================= SE out proj (fm) ==============================
        xs_t = [pxs.tile([128, R], F32R, tag="xs", name=f"xs{m}") for m in range(NT)]
        wse_t = load_w(wse[:], "t_wse")
        for mt in range(NT):
            for c in range(GPC):
                ps = pp.tile([128, L], F32, tag="ps", name=f"ps_se{mt}_{c}")
                for kt in range(NT):
                    nc.tensor.matmul(
                        out=ps[:],
                        lhsT=_r(wse_t[:, kt, 128 * mt:128 * (mt + 1)]),
                        rhs=_r(h3_t[kt][:, L * c:L * (c + 1)]),
                        start=(kt == 0), stop=(kt == NT - 1), skip_group_check=True)
                nc.scalar.activation(
                    out=xs_t[mt][:, L * c:L * (c + 1)], in_=ps[:], func=AF.Identity,
                    bias=ct_bpack[:, BP_SE + mt:BP_SE + mt + 1])
        ph3.release()

        # ======== QK (per graph, fm) + attention + output proj ===========
        pqk = tc.alloc_tile_pool(name="qk", bufs=4, side="right")
        wqa_t = load_w(wqk[:, 0:D], "t_wqa")
        wqb_t = load_w(wqk[:, D:2 * D], "t_wqb")

        pv = tc.alloc_tile_pool(name="vg", bufs=2, side="right")
        pm = tc.alloc_tile_pool(name="mg", bufs=1, side="right")
        pe = tc.alloc_tile_pool(name="e", bufs=5, side="right")
        pusb = tc.alloc_tile_pool(name="usb", bufs=10, side="right")
        pds = tc.alloc_tile_pool(name="dsb", bufs=6, side="right")
        prec = tc.alloc_tile_pool(name="rec", bufs=4, side="right")
        pwo = tc.alloc_tile_pool(name="wo", bufs=8, side="right")
        py = tc.alloc_tile_pool(name="y", bufs=1, side="right")

        wo_t = []
        for h in range(H):
            t = pwo.tile([128, D], F32R, tag="wo", name=f"wo{h}")
            nc.sync.dma_start(out=t[0:HD, :], in_=wo[HD * h:HD * (h + 1), :])
            wo_t.append(t)

        for g in range(GPC):
            kt_g = pqk.tile([128, NT, L], F32R, tag="qk", name=f"kt{g}")
            qt_g = pqk.tile([128, NT, L], F32R, tag="qk", name=f"qt{g}")
            for mt in range(2 * NT):
                wt = wqa_t if mt < NT else wqb_t
                dst = kt_g if mt < NT else qt_g
                ps = pp.tile([128, L], F32, tag="ps", name=f"ps_qk{g}_{mt}")
                for kk in range(NT):
                    nc.tensor.matmul(
                        out=ps[:],
                        lhsT=_r(wt[:, kk, 128 * (mt % NT):128 * (mt % NT + 1)]),
                        rhs=_r(xs_t[kk][:, L * g:L * (g + 1)]),
                        start=(kk == 0), stop=(kk == NT - 1), skip_group_check=True)
                nc.scalar.activation(
                    out=dst[:, mt % NT, :], in_=ps[:], func=AF.Identity,
                    bias=ct_bpack[:, BP_QK + mt:BP_QK + mt + 1])
            vg = pv.tile([128, NT, L], F32R, tag="vg", name=f"vg{g}")
            nc.sync.dma_start(
                out=vg[:],
                in_=v_dram[L * g:L * (g + 1), :].rearrange("(st p) n -> p st n", p=128))
            mg = pm.tile([128, NT, L], F32R, tag="mg", name=f"mg{g}")
            nc.sync.dma_start(
                out=mg[:],
                in_=mT[L * g:L * (g + 1), :].rearrange("(st p) n -> p st n", p=128))
            usb_g = [None] * H
            for hp in range(H // 2):
                # head pair (2hp, 2hp+1): K=64 dots matmuls target disjoint
                # PE row strips (partitions 0:64 / 64:128) and are emitted
                # back-to-back so the array runs them concurrently.
                ups, dns = [], []
                for sub in range(2):
                    ups.append(pp.tile([128, L], F32, tag="ps", name=f"pu{g}_{hp}_{sub}"))
                    dns.append(pp.tile([128, L], F32, tag="ps", name=f"pn{g}_{hp}_{sub}"))
                ets = [[None] * NT for _ in range(2)]
                for jt in range(NT):
                    dpss = []
                    for sub in range(2):
                        off = 64 * sub
                        dps = pp.tile([128, L], F32, tag="ps", name=f"pd{g}_{hp}_{sub}_{jt}")
                        nc.tensor.matmul(
                            out=dps[:],
                            lhsT=_r(kt_g[off:off + HD, hp, 128 * jt:128 * (jt + 1)]),
                            rhs=_r(qt_g[off:off + HD, hp, :]),
                            start=True, stop=True, skip_group_check=True)
                        dpss.append(dps)
                    for sub in range(2):
                        h = 2 * hp + sub
                        ep = pe.tile([128, L], F32, tag="ep", name=f"ep{g}_{h}_{jt}")
                        nc.vector.tensor_add(out=ep[:], in0=dpss[sub][:], in1=mg[:, jt, :])
                        et = pe.tile([128, L], F32R, tag="e", name=f"e{g}_{h}_{jt}")
                        nc.scalar.activation(out=et[:], in_=ep[:], func=AF.Exp)
                        nc.tensor.matmul(
                            out=ups[sub][0:HD, :],
                            lhsT=_r(vg[:, jt, HD * h:HD * (h + 1)]), rhs=_r(et[:]),
                            start=(jt == 0), stop=(jt == NT - 1), skip_group_check=True)
                        nc.tensor.matmul(
                            out=dns[sub][0:1, :], lhsT=_r(ct_ones[:, 0:1]), rhs=_r(et[:]),
                            start=(jt == 0), stop=(jt == NT - 1), skip_group_check=True)
                for sub in range(2):
                    h = 2 * hp + sub
                    up, dn = ups[sub], dns[sub]
                    usb = pusb.tile([128, L], F32R, tag="usb", name=f"usb{g}_{h}")
                    dsb = pds.tile([1, L], F32R, tag="dsb", name=f"dsb{g}_{h}")
                    nc.any.tensor_copy(out=usb[0:HD, :], in_=up[0:HD, :])
                    nc.any.tensor_copy(out=dsb[0:1, :], in_=dn[0:1, :])
                    # broadcast denominator (reuses dn's bank; start=True
                    # clears it after the dsb eviction).
                    nc.tensor.matmul(
                        out=dn[0:HD, :], lhsT=_r(ct_ones[0:1, 0:HD]),
                        rhs=_r(dsb[0:1, :]),
                        start=True, stop=True, skip_group_check=True)
                    rb = prec.tile([HD, L], F32, tag="rb", name=f"rb{g}_{h}")
                    sc = prec.tile([HD, L], F32, tag="rb", name=f"sc{g}_{h}")
                    nc.vector.reciprocal_approx_accurate(
                        out=rb[0:HD, :], in_=dn[0:HD, :], scratch=sc[0:HD, :])
                    nc.vector.tensor_mul(
                        out=usb[0:HD, :], in0=usb[0:HD, :], in1=rb[0:HD, :])
                    usb_g[h] = usb
            yg = py.tile([128, NT, D], F32, tag="y", name=f"y{g}")
            for rt in range(NT):
                ps = pp.tile([128, D], F32, tag="ps", name=f"ps_y{g}_{rt}")
                for h in range(H):
                    nc.tensor.matmul(
                        out=ps[:],
                        lhsT=_r(usb_g[h][0:HD, 128 * rt:128 * (rt + 1)]),
                        rhs=_r(wo_t[h][0:HD, :]),
                        start=(h == 0), stop=False, skip_group_check=True)
                bias_mm(ps[:], BR_OUTB)
                nc.any.tensor_copy(out=yg[:, rt, :], in_=ps[:])
            nc.sync.dma_start(
                out=y[L * g:L * (g + 1), :].rearrange("(st p) n -> p st n", p=128),
                in_=yg[:])

        # release: reverse-alloc order per side (stack allocator)
        for p in (py, pwo, prec, pds, pusb, pe, pm, pv, pqk):
            p.release()  # right side
        for p in (pxs, pw, pcon):
            p.release()  # left side (ph3 already released after SE)
        pp.release()
        pd.release()
    nc.compile()
    return nc


@functools.lru_cache(maxsize=1)
def _program(inv_n: float, zflags: tuple):
    nc, di, y = _build()
    return _emit(nc, di, y, inv_n, zflags)


def _prep_core(c, x, ptr, mask_dag, weights):
    """Build the per-core input map (numpy only; sharding/layout prep)."""
    xs = np.zeros((R, D), np.float32)
    cpm = np.zeros((R, L), np.float32)
    mTm = np.zeros((R, L), np.float32)
    validm = np.zeros((R,), np.float32)
    for gi in range(GPC):
        g = GPC * c + gi
        s0, s1 = int(ptr[g]), int(ptr[g + 1])
        S = s1 - s0
        xs[L * gi:L * gi + S] = x[s0:s1]
        validm[L * gi:L * gi + S] = 1.0
        # adjacency + identity (dst j <- src s), local indices
        cpm[L * gi:L * gi + L, :] += np.eye(L, dtype=np.float32)
        cpm[L * gi:L * gi + S, :S] += weights["adj"][g][:S, :S]
        # additive mask, transposed to [key j, query i]
        md = mask_dag[g]  # [L, L] bool, True = masked
        pad_key = np.zeros((L,), bool)
        pad_key[S:] = True
        m = np.where(md | pad_key[None, :], NEG, 0.0).astype(np.float32)
        mTm[L * gi:L * gi + L, :] = m.T
    return {
        "x_rm": np.ascontiguousarray(xs),
        "xT": np.ascontiguousarray(xs.T),
        "cp": cpm,
        "mT": mTm,
        "wg1": weights["wg1"], "wg2": weights["wg2"],
        "wse": weights["wse"], "wqk": weights["wqk"],
        "wv": weights["wv"], "wo": weights["wo"],
        "brow": weights["brow"], "bpack": weights["bpack"],
        "valid": np.ascontiguousarray(np.broadcast_to(validm, (128, R))),
        "ident": np.eye(128, dtype=np.float32),
        "onem": np.ones((128, 128), np.float32),
    }


def _pack_col(vec):  # [512] -> [128, 4] (fm per-partition bias layout)
    return np.asarray(vec, np.float32).reshape(NT, 128).T


def _host_prep(inputs):
    x = np.asarray(inputs["x"], np.float32)
    ptr = np.asarray(inputs["ptr"], np.int64)
    mask_dag = np.asarray(inputs["mask_dag_"], bool)
    ei = np.asarray(inputs["edge_index"], np.int64)
    N = int(ptr[-1])

    # dense per-graph adjacency counts: adj[g][dst_local, src_local]
    src, dst = ei[0], ei[1]
    gid = np.searchsorted(ptr, dst, side="right") - 1
    adj = [np.zeros((L, L), np.float32) for _ in range(B)]
    ls = src - ptr[gid]
    ld = dst - ptr[gid]
    for g in range(B):
        m = gid == g
        np.add.at(adj[g], (ls[m], ld[m]), 1.0)  # [src s, dst j] = count(s->j)

    scale = np.float32(1.0 / np.sqrt(D // H))
    wqk = np.asarray(inputs["qk_w"], np.float32).copy()
    qkb = np.asarray(inputs["qk_b"], np.float32).copy()
    wqk[:, D:] *= scale
    qkb[D:] *= scale

    bpack = np.zeros((128, BP_NCOL), np.float32)
    for lay in range(3):
        bpack[:, BP_B1 + 4 * lay:BP_B1 + 4 * lay + NT] = _pack_col(inputs["gin_b1"][lay])
    bpack[:, BP_B2L2:BP_B2L2 + NT] = _pack_col(inputs["gin_b2"][2])
    bpack[:, BP_SE:BP_SE + NT] = _pack_col(inputs["se_out_b"])
    bpack[:, BP_QK:BP_QK + 2 * NT] = np.asarray(qkb, np.float32).reshape(2 * NT, 128).T
    bpack[:, BP_GAMMA:BP_GAMMA + NT] = _pack_col(inputs["bn_gamma"])
    bpack[:, BP_BETA:BP_BETA + NT] = _pack_col(inputs["bn_beta"])

    brow = np.zeros((1, 4, D), np.float32)
    brow[0, BR_VB] = np.asarray(inputs["v_b"], np.float32)
    brow[0, BR_B2L0] = np.asarray(inputs["gin_b2"][0], np.float32)
    brow[0, BR_B2L1] = np.asarray(inputs["gin_b2"][1], np.float32)
    brow[0, BR_OUTB] = np.asarray(inputs["out_b"], np.float32)

    weights = {
        "adj": adj,
        "wg1": np.ascontiguousarray(inputs["gin_w1"], dtype=np.float32),
        "wg2": np.ascontiguousarray(inputs["gin_w2"], dtype=np.float32),
        "wse": np.ascontiguousarray(inputs["se_out_w"], dtype=np.float32),
        "wqk": np.ascontiguousarray(wqk),
        "wv": np.ascontiguousarray(inputs["v_w"], dtype=np.float32),
        "wo": np.ascontiguousarray(inputs["out_w"], dtype=np.float32),
        "brow": brow, "bpack": bpack,
    }
    in_maps = [_prep_core(c, x, ptr, mask_dag, weights) for c in range(NCORE)]
    return in_maps, N, ptr


def _ensure_ntff_hook():
    """The agent image's antenv lacks axon_hooks; synthesize it and register
    the boot shim's ctypes NTFF profiler so trace=True works."""
    import types
    try:
        from antenv.axon_hooks import get_axon_ntff_profile_hook  # noqa: F401
        return
    except ImportError:
        pass
    mod = types.ModuleType("antenv.axon_hooks")
    _hook = [None]
    mod.set_axon_ntff_profile_hook = lambda h: _hook.__setitem__(0, h)
    mod.get_axon_ntff_profile_hook = lambda: _hook[0]
    sys.modules["antenv.axon_hooks"] = mod
    import antenv
    antenv.axon_hooks = mod
    try:
        if "/root/.axon_site" not in sys.path:
            sys.path.insert(0, "/root/.axon_site")
        from trn_agent_boot.trn_boot import _ntff_profile_via_ctypes
        mod.set_axon_ntff_profile_hook(
            _ntff_profile_via_ctypes("/opt/axon/libaxon_pjrt.so"))
    except Exception as e:  # degrade to no-trace
        print("ntff hook unavailable:", e)


def run(inputs, trace=False):
    if trace:
        _ensure_ntff_hook()
    in_maps, N, ptr = _host_prep(inputs)
    br = in_maps[0]["brow"]
    zflags = tuple(bool(not np.any(br[0, s])) for s in range(4))
    nc = _program(float(1.0 / N), zflags)
    res = run_bass_kernel_spmd(
        nc, in_maps, core_ids=list(range(NCORE)), trace=trace)
    out = np.empty((N, D), np.float32)
    for c in range(NCORE):
        yc = res.results[c]["y"]
        for gi in range(GPC):
            g = GPC * c + gi
            s0, s1 = int(ptr[g]), int(ptr[g + 1])
            out[s0:s1] = yc[L * gi:L * gi + (s1 - s0)]
    return out, res


def kernel(**inputs):
    out, _ = run(inputs, trace=False)
    return out


if __name__ == "__main__":
    pass

